# revision 41
# baseline (speedup 1.0000x reference)
"""Trainium2 Bass kernel for nn_MultiHeadEncDecAttention.

Problem (full shapes):
  x:[4,512,8,256] z:[256,512,32] w_q_w:[256,256] fc_w:[256,256] (+biases, LN params)
  q = x@w_q_w.T (+b) -> [h,v,b,s,dq]; attn = softmax(q@z^T/sqrt(dq)); out = attn@z
  o2 = concat_h(out)@fc_w.T (+b); y = LN(o2 + x)*gamma + beta

Sharding: split on n_verts (nv=8) across the 8 cores - every stage
(q-proj, attention, fc, LN) is independent per vert, so zero cross-core comms.

v3 design (per core, r = b*512+s in [0,2048)):
  Same pipeline skeleton as v2, but the softmax exp - the single biggest
  serial cost (64 ACT ops ~ 90us) - is split across TWO engines:
    - ACT path: table exp, exact. zT is host-prescaled by log2e/sqrt(dq),
      so ACT uses scale=ln2 (exp(t*ln2) = 2^t).
    - DVE path: Schraudolph bit-trick exp in ONE tensor_scalar:
      i16 = rint(128*t + 16254); reinterpret as bf16 == 2^t * (1 +- 3%).
      The +-3% per-weight error cancels in the softmax normalization;
      end-to-end rel err ~ 0.005 (tolerance 2e-2).
  Engine rebalance: qproj/AV casts and sums-cast move ACT-ward (Copy),
  the LN apply moves to GPSIMD, bn_stats/recip/norm-mult stay DVE.
"""

import sys

sys.path.insert(0, "/opt/trn_rl_repo")

from contextlib import ExitStack

import ml_dtypes
import numpy as np

import concourse.bass as bass
import concourse.tile as tile
from concourse import mybir

F32 = mybir.dt.float32
BF16 = mybir.dt.bfloat16
I16 = mybir.dt.int16
AX = mybir.AluOpType
AF = mybir.ActivationFunctionType

N_HEAD = 8
D_Q = 32
D_IN = 256
BS = 4
SEG = 512
NV = 8
LN_EPS = 1e-5
R = BS * SEG  # 2048 rows per core
NCORES = 8
INV_TEMP = 1.0 / np.sqrt(np.float32(D_Q))
LOG2E = 1.4426950408889634
LN2 = 0.6931471805599453
H_PERM = [0, 4, 1, 5, 2, 6, 3, 7]  # head for order index o

# exp engine split: flat unit idx u = (b*8+o)*2+th in [0,64). True -> DVE
# Schraudolph, False -> ACT table exp. ~26/64 on DVE.
DVE_EXP = [u % 8 in (1, 4, 6) or u == 3 for u in range(64)]

_prog_cache = {}


def _build(use_wqb: bool, use_gb: bool):
    from concourse import bacc

    nc = bacc.Bacc("TRN2", target_bir_lowering=False, debug=False)

    d_xT = nc.dram_tensor("xT", [2, 128, R], BF16, kind="ExternalInput").ap()
    d_xres = nc.dram_tensor("xres", [128, 16 * 256], F32, kind="ExternalInput").ap()
    d_zT = nc.dram_tensor("zT", [2, 128, 2048], BF16, kind="ExternalInput").ap()
    d_zA = nc.dram_tensor("zA", [4, 128, 66 * 16], BF16, kind="ExternalInput").ap()
    d_wqT = nc.dram_tensor("wqT", [2, 128, 256], BF16, kind="ExternalInput").ap()
    d_wqb = nc.dram_tensor("wqb", [128, 2], F32, kind="ExternalInput").ap()
    d_fcT = nc.dram_tensor("fcT", [4, 128, D_IN], BF16, kind="ExternalInput").ap()
    d_gbb = nc.dram_tensor("gbb", [128, 512], F32, kind="ExternalInput").ap()
    d_ind8 = nc.dram_tensor("ind8", [8, 512], BF16, kind="ExternalInput").ap()
    d_y = nc.dram_tensor("y", [R, D_IN], F32, kind="ExternalOutput").ap()

    with tile.TileContext(nc) as tc, ExitStack() as ctx:
        P = ctx.enter_context  # noqa

        big = P(tc.tile_pool(name="big", bufs=1))
        lgp = P(tc.tile_pool(name="lgp", bufs=3, space="PSUM"))
        avp = P(tc.tile_pool(name="avp", bufs=2, space="PSUM"))
        expp = P(tc.tile_pool(name="expp", bufs=6))
        smp = P(tc.tile_pool(name="smp", bufs=2))
        stp = P(tc.tile_pool(name="stp", bufs=2))
        outp = P(tc.tile_pool(name="outp", bufs=3))

        # ---- persistent SBUF tiles + input DMAs
        eps_t = big.tile([128, 1], F32)
        nc.vector.memset(eps_t[:], float(LN_EPS))
        dummy_t = big.tile([128, 1], F32)
        # early Exp so the ACT table set loads during the DMA phase
        nc.scalar.activation(dummy_t[:], eps_t[:], AF.Exp)

        # tile for the PE warm-up burst
        warm_t = big.tile([128, 512], BF16, name="warm")
        nc.gpsimd.memset(warm_t[:], 0.0)

        # Inputs are tiled per-batch and DMA'd in first-use order, so batch
        # 0's attention starts after ~0.9 MiB instead of the full ~6 MiB.
        wqT_t = [big.tile([128, 256], BF16, name=f"wqT{k}") for k in range(2)]
        # xT as separate per-chunk tiles so qproj chunk n depends only on its
        # own DMA (dep tracking for DMA writes is whole-tile)
        xT_t = [
            [big.tile([128, 512], BF16, name=f"xT{k}_{n}") for n in range(4)]
            for k in range(2)
        ]
        zT_t = [
            [big.tile([128, 512], BF16, name=f"zT{u}_{b}") for b in range(BS)]
            for u in range(2)
        ]
        zA_t = [
            [big.tile([128, 66 * 4], BF16, name=f"zA{c}_{b}") for b in range(BS)]
            for c in range(4)
        ]
        wqb_t = big.tile([128, 2], F32)
        ind8_t = big.tile([8, 512], BF16)
        fcT_t = [big.tile([128, D_IN], BF16, name=f"fcT{e}") for e in range(4)]
        gbb_t = big.tile([128, 512], F32)
        xres_t = big.tile([128, 16 * 256], F32)

        # gpsimd queue: batch-0 critical path first, then per-batch z data
        for k in range(2):
            nc.gpsimd.dma_start(xT_t[k][0][:], d_xT[k, :, 0:512])
        for u in range(2):
            nc.gpsimd.dma_start(zT_t[u][0][:], d_zT[u, :, 0:512])
        for c in range(4):
            nc.gpsimd.dma_start(zA_t[c][0][:], d_zA[c, :, 0:264])
        for b in range(1, BS):
            for u in range(2):
                nc.gpsimd.dma_start(zT_t[u][b][:], d_zT[u, :, 512 * b : 512 * (b + 1)])
            for c in range(4):
                nc.gpsimd.dma_start(zA_t[c][b][:], d_zA[c, :, 264 * b : 264 * (b + 1)])
        # sync queue: weights, remaining x chunks, residual
        for k in range(2):
            nc.sync.dma_start(wqT_t[k][:], d_wqT[k])
        if use_wqb:
            nc.sync.dma_start(wqb_t[:], d_wqb)
        for k in range(2):
            nc.sync.dma_start(xT_t[k][1][:], d_xT[k, :, 512:1024])
        nc.sync.dma_start(ind8_t[:], d_ind8)
        for e in range(4):
            nc.sync.dma_start(fcT_t[e][:], d_fcT[e])
        for n in range(2, 4):
            for k in range(2):
                nc.sync.dma_start(xT_t[k][n][:], d_xT[k, :, 512 * n : 512 * (n + 1)])
        if use_gb:
            nc.sync.dma_start(gbb_t[:], d_gbb)
        nc.sync.dma_start(xres_t[:], d_xres)

        qT_t = [big.tile([128, R], BF16, name=f"qT{u}") for u in range(2)]
        # outcT tile k: head k rows 0:32, sums row 32; head k+4 rows 64:96,
        # sums row 96; rows 33:63 / 97:127 junk (zeroed by the norm multiply)
        outcT = [big.tile([128, R], BF16, name=f"outcT{e}") for e in range(4)]
        yhold = big.tile([128, 16 * 256], F32)
        mvall = big.tile([128, 32], F32)

        def mm(out, lhsT, rhs, **kw):
            nc.tensor.matmul(out, lhsT, rhs, skip_group_check=True, **kw)

        # zero the never-matmul-written rows of the two av PSUM slots once,
        # so the full-partition cast below never reads non-finite stale PSUM
        for _ in range(2):
            av0 = avp.tile([128, 512], F32, tag="avb", name="av_init")
            nc.vector.memset(av0[32:64, :], 0.0)
            nc.vector.memset(av0[96:128, :], 0.0)

        # PE warm-up burst: dummy matmuls on junk (zero) data with no input
        # deps keep the HAM activity window busy. 8 up front; the rest are
        # sprinkled between the first unit pairs (on the avp pool, writing
        # zeros, so real logits slots are not delayed behind them).
        for w in range(8):
            wp = lgp.tile([128, 512], F32, tag="lg", name="warmmm")
            mm(wp[:], warm_t[:, 0:128], warm_t[:], start=True, stop=True)

        def dummy_avp(n):
            for _ in range(n):
                wp = avp.tile([128, 512], F32, tag="avb", name="dummy")
                mm(wp[:], warm_t[:, 0:128], warm_t[:], start=True, stop=True)

        # ---- q projection chunk: qT[tile T][:, 512n:512n+512]
        def emit_qproj(T, n):
            qp = avp.tile([128, 512], F32, tag="avb", name="qp")
            for k in range(2):
                mm(
                    qp[:],
                    wqT_t[k][:, 128 * T : 128 * (T + 1)],
                    xT_t[k][n][:],
                    start=(k == 0),
                    stop=(k == 1),
                )
            dst = qT_t[T][:, 512 * n : 512 * (n + 1)]
            if use_wqb:
                nc.vector.tensor_scalar(
                    dst, qp[:], wqb_t[:, T : T + 1], 0.0, AX.add, AX.add
                )
            elif n == 0:
                # batch-0 fill phase: DVE is idle, ACT is the exp engine
                nc.vector.tensor_copy(dst, qp[:])
            else:
                nc.scalar.activation(dst, qp[:], AF.Copy)

        # ---- logits + exp for one (order-index o, batch b, t-half th) unit
        expt_tiles = {}

        def _exp_of(b, o, th, lt):
            u = (b * 8 + o) * 2 + th
            expt = expt_tiles[(b, o)]
            dst = expt[:, 1024 * th : 1024 * (th + 1)]
            if DVE_EXP[u]:
                # Schraudolph: i16 = rint(128*t + 16254); bits == bf16 2^t
                nc.vector.tensor_scalar(
                    dst.bitcast(I16), lt[:], 128.0, 16254.0, AX.mult, AX.add
                )
            else:
                nc.scalar.activation(dst, lt[:], AF.Exp, scale=float(LN2))

        def emit_logits_exp_pair(b, g, th):
            # both units (o=2g, 2g+1) of one t-half, logits mms interleaved
            # across the two row-bands so consecutive PE matmuls overlap
            lts = {}
            for o in (2 * g, 2 * g + 1):
                if th == 0:
                    expt_tiles[(b, o)] = expp.tile([128, 2048], BF16, name="expt")
                lts[o] = lgp.tile([128, 1024], F32, tag="lg", name="lt")
            for j in range(2):
                c = 2 * th + j
                for o in (2 * g, 2 * g + 1):
                    T, beta = o // 4, 32 * (o % 4)
                    mm(
                        lts[o][:, 512 * j : 512 * (j + 1)],
                        zT_t[T][b][beta : beta + 32, 128 * c : 128 * (c + 1)],
                        qT_t[T][beta : beta + 32, 512 * b : 512 * (b + 1)],
                        start=True,
                        stop=True,
                        tile_position=(beta, 0),
                    )
            for o in (2 * g, 2 * g + 1):
                _exp_of(b, o, th, lts[o])

        # ---- AV for av-pair g of batch b (heads g and g+4, col-tiled {0,64})
        # split into two emission halves so the matmuls interleave between
        # later logits units (AV of group G-2 never waits on anything)
        av_state = {}

        def emit_av_half(G, half):
            b, g = divmod(G, 4)
            q = 4 * b + g
            if half == 0:
                av_state[G] = (
                    avp.tile([128, 512], F32, tag="avb", name="av"),
                    expt_tiles.pop((b, 2 * g)),
                    expt_tiles.pop((b, 2 * g + 1)),
                )
            av, eA, eB = av_state[G]
            for c in (2 * half, 2 * half + 1):
                mm(
                    av[0:33, :],
                    zA_t[c][b][:, 66 * g : 66 * g + 33],
                    eA[:, 512 * c : 512 * (c + 1)],
                    start=(c == 0),
                    stop=(c == 3),
                )
                mm(
                    av[64:97, :],
                    zA_t[c][b][:, 66 * g + 33 : 66 * g + 66],
                    eB[:, 512 * c : 512 * (c + 1)],
                    start=(c == 0),
                    stop=(c == 3),
                )
            if half == 1:
                dst = outcT[g][:, 512 * b : 512 * (b + 1)]
                if b == 0:
                    nc.vector.tensor_copy(dst, av[:])
                else:
                    nc.scalar.activation(dst, av[:], AF.Copy)
                del av_state[G]

        # ---- epilogue pieces for batch b
        sums_tiles = {}

        def emit_sums_dma(b, ks=(0, 1, 2, 3)):
            if b not in sums_tiles:
                sums_tiles[b] = smp.tile([8, 512], BF16, tag="sums", name="sums_b")
            sums_b = sums_tiles[b]
            for k in ks:
                src = outcT[k][32:97:64, 512 * b : 512 * (b + 1)]
                eng = nc.sync if k % 2 == 0 else nc.gpsimd
                eng.dma_start(sums_b[2 * k : 2 * k + 2, :], src)

        B_tiles = {}
        recip_state = {}

        def emit_norm_recip(b, r0=0, r1=8):
            sums_b = sums_tiles[b]
            if b not in recip_state:
                recip_state[b] = (
                    smp.tile([8, 512], F32, name="sumf"),
                    smp.tile([8, 512], F32, name="recf"),
                    smp.tile([8, 512], BF16, name="recb"),
                )
            sumf, recf, recb = recip_state[b]
            nc.scalar.activation(sumf[r0:r1, :], sums_b[r0:r1, :], AF.Copy)
            nc.vector.reciprocal_approx_fast(recf[r0:r1, :], sumf[r0:r1, :])
            nc.vector.tensor_copy(recb[r0:r1, :], recf[r0:r1, :])
            B_tiles[b] = recb
            if r1 == 8:
                del recip_state[b]
                del sums_tiles[b]

        tail3 = {}

        def emit_tail_sums3(b):
            # k=3 sums into a separate partition-0-aligned [2,512] tile
            sumsB = smp.tile([2, 512], BF16, name="sumsB")
            nc.gpsimd.dma_start(sumsB[:], outcT[3][32:97:64, 512 * b : 512 * (b + 1)])
            tail3["sums"] = sumsB

        def emit_tail_norm3(b):
            sumfB = smp.tile([2, 512], F32, name="sumfB")
            nc.vector.tensor_copy(sumfB[:], tail3["sums"][:])
            recfB = smp.tile([2, 512], F32, name="recfB")
            nc.vector.reciprocal_approx_fast(recfB[:], sumfB[:])
            recbB = smp.tile([2, 512], BF16, name="recbB")
            nc.vector.tensor_copy(recbB[:], recfB[:])
            # ind8's k=0 block has exactly the right row pattern (0:32, 64:96)
            Bt = avp.tile([128, 512], F32, tag="avb", name="Bt3")
            mm(Bt[:], ind8_t[0:2, 0:128], recbB[:], start=True, stop=True)
            sl = outcT[3][:, 512 * b : 512 * (b + 1)]
            nc.vector.tensor_tensor(sl, sl, Bt[:], AX.mult)

        def emit_norm_mult(b, k):
            recb = B_tiles[b]
            Bt = avp.tile([128, 512], F32, tag="avb", name="Bt")
            mm(Bt[:], ind8_t[:, 128 * k : 128 * (k + 1)], recb[:], start=True, stop=True)
            sl = outcT[k][:, 512 * b : 512 * (b + 1)]
            nc.vector.tensor_tensor(sl, sl, Bt[:], AX.mult)

        def emit_fc_chunk(b, sc):
            ci = 4 * b + sc
            reg = avp.tile([128, 512], F32, tag="avb", name="fcp")[:, 0:256]
            for k in range(4):
                mm(
                    reg[:],
                    outcT[k][:, 512 * b + 128 * sc : 512 * b + 128 * (sc + 1)],
                    fcT_t[k][:],
                    start=(k == 0),
                    stop=(k == 3),
                )
            ysl = yhold[:, 256 * ci : 256 * (ci + 1)]
            nc.vector.tensor_tensor(
                ysl, reg[:], xres_t[:, 256 * ci : 256 * (ci + 1)], AX.add
            )
            st6 = stp.tile([128, 6], F32, name="st6")
            nc.vector.bn_stats(st6[:], ysl)
            nc.vector.bn_aggr(mvall[:, 2 * ci : 2 * ci + 2], st6[:])

        def emit_apply(b, sc, y_ap, nmr_ap):
            # yo = (ysl - mu) * rstd, on ACT: Identity(ysl*rstd + (-mu*rstd))
            ci = 4 * b + sc
            ysl = yhold[:, 256 * ci : 256 * (ci + 1)]
            yo = outp.tile([128, 256], F32, name="yo")
            if use_gb:
                t2 = outp.tile([128, 256], F32, tag="t1", name="t2")
                nc.vector.scalar_tensor_tensor(
                    t2[:], ysl, mvall[:, 2 * ci : 2 * ci + 1], gbb_t[:, 0:256],
                    AX.subtract, AX.mult,
                )
                nc.vector.scalar_tensor_tensor(
                    yo[:], t2[:], y_ap, gbb_t[:, 256:512],
                    AX.mult, AX.add,
                )
            else:
                nc.scalar.activation(
                    yo[:], ysl, AF.Identity, bias=nmr_ap, scale=y_ap
                )
            eng = nc.gpsimd if sc % 2 == 0 else nc.sync
            eng.dma_start(d_y[128 * ci : 128 * (ci + 1), :], yo[:])

        def _newton_rstd(va, n):
            y = stp.tile([128, n], F32, tag=f"ny{n}", name="ny")
            nc.vector.reciprocal_approx_fast(y[:], va[:])
            for _ in range(3):
                t1 = stp.tile([128, n], F32, tag=f"nt{n}", name="nt1")
                nc.vector.tensor_tensor(t1[:], y[:], y[:], AX.mult)
                nc.vector.tensor_tensor(t1[:], t1[:], va[:], AX.mult)
                nc.vector.tensor_scalar(t1[:], t1[:], -0.5, 1.5, AX.mult, AX.add)
                nc.vector.tensor_tensor(y[:], y[:], t1[:], AX.mult)
            return y

        def emit_rstd_all(b):
            # rstd = 1/sqrt(var+eps) on DVE (recip-approx seed + 3 Newton
            # rsqrt iterations) - avoids the ACT sqrt table switch entirely.
            mvb = mvall[:, 8 * b : 8 * (b + 1)].rearrange("p (c two) -> p c two", two=2)
            va = stp.tile([128, 4], F32, tag="va4", name="va")
            nc.vector.tensor_scalar(va[:], mvb[:, :, 1:2], eps_t[:], 0.0, AX.add, AX.add)
            y = _newton_rstd(va, 4)
            # nmr = (-mu) * rstd (per-chunk bias for the ACT apply)
            nmr = stp.tile([128, 4], F32, tag="nm4", name="nmr")
            nc.vector.scalar_tensor_tensor(
                nmr[:], mvb[:, :, 0:1], -1.0, y[:], AX.mult, AX.mult
            )
            return y, nmr

        def emit_rstd_sqrt(b, cols):
            # tail-only rstd: ACT Sqrt(var+eps) (sqrt table set - only legal
            # after the LAST Exp op) + DVE fast reciprocal. ~4 ops total vs
            # a ~13-op Newton chain.
            n = len(cols)
            c0 = cols[0]
            assert cols == list(range(c0, c0 + n))
            mvb = mvall[:, 8 * b : 8 * (b + 1)].rearrange("p (c two) -> p c two", two=2)
            var_ap = mvb[:, c0 : c0 + n, 1:2]
            mu_ap = mvb[:, c0 : c0 + n, 0:1]
            sqv = stp.tile([128, n], F32, tag=f"sq{n}", name="sqv")
            nc.scalar.activation(sqv[:], var_ap, AF.Sqrt, bias=eps_t[:, 0:1])
            y = stp.tile([128, n], F32, tag=f"sy{n}", name="sy")
            nc.vector.reciprocal_approx_fast(y[:], sqv[:])
            nmr = stp.tile([128, n], F32, tag=f"sm{n}", name="smr")
            nc.vector.scalar_tensor_tensor(
                nmr[:], mu_ap, -1.0, y[:], AX.mult, AX.mult
            )
            return y, nmr

        def emit_ln_sqrt(b, cols):
            y, nmr = emit_rstd_sqrt(b, cols)
            for j, sc in enumerate(cols):
                emit_apply(b, sc, y[:, j : j + 1], nmr[:, j : j + 1])

        def emit_ln_sqrt_tail(b, sc):
            # tail chunk: sqrt -> fast recip -> DVE apply (shortest chain to
            # the final output DMA; DVE is idle in the tail)
            if use_gb:
                emit_ln_sqrt(b, [sc])
                return
            ci = 4 * b + sc
            sqv = stp.tile([128, 1], F32, tag="sq1", name="sqv1")
            nc.scalar.activation(
                sqv[:], mvall[:, 2 * ci + 1 : 2 * ci + 2], AF.Sqrt,
                bias=eps_t[:, 0:1],
            )
            y = stp.tile([128, 1], F32, tag="sy1", name="sy1")
            nc.vector.reciprocal_approx_fast(y[:], sqv[:])
            ysl = yhold[:, 256 * ci : 256 * (ci + 1)]
            yo = outp.tile([128, 256], F32, name="yo")
            nc.vector.tensor_scalar(
                yo[:], ysl, mvall[:, 2 * ci : 2 * ci + 1], y[:, 0:1],
                AX.subtract, AX.mult,
            )
            eng = nc.gpsimd if sc % 2 == 0 else nc.sync
            eng.dma_start(d_y[128 * ci : 128 * (ci + 1), :], yo[:])

        def emit_ln_finish(b):
            y, nmr = emit_rstd_all(b)
            for sc in range(4):
                emit_apply(b, sc, y[:, sc : sc + 1], nmr[:, sc : sc + 1])

        # one merged Newton rstd chain for batches 0 and 1 ([128, 8]),
        # emitted once batch 1's bn stats are all aggregated
        chain01 = {}

        def emit_rstd01():
            mvb = mvall[:, 0:16].rearrange("p (c two) -> p c two", two=2)
            va = stp.tile([128, 8], F32, tag="va8", name="va8")
            nc.vector.tensor_scalar(va[:], mvb[:, :, 1:2], eps_t[:], 0.0, AX.add, AX.add)
            y = _newton_rstd(va, 8)
            nmr = stp.tile([128, 8], F32, tag="nm8", name="nmr8")
            nc.vector.scalar_tensor_tensor(
                nmr[:], mvb[:, :, 0:1], -1.0, y[:], AX.mult, AX.mult
            )
            chain01["y"] = y
            chain01["nmr"] = nmr

        def emit_apply01(b):
            y, nmr = chain01["y"], chain01["nmr"]
            for sc in range(4):
                j = 4 * b + sc
                emit_apply(b, sc, y[:, j : j + 1], nmr[:, j : j + 1])

        # ---- main schedule -------------------------------------------------
        # qproj for batch 0 first (only needs wqT + xT chunk 0)
        emit_qproj(0, 0)
        emit_qproj(1, 0)
        # bridge dummies: keep the PE busy between qproj and the first logits
        dummy_avp(4)

        def emit_av(b, g):
            emit_av_half(4 * b + g, 0)
            emit_av_half(4 * b + g, 1)

        # epilogue pieces of batch b-1, spread across batch b's groups.
        # LN applies for batches 0/1 are deferred to the merged chain01
        # (emitted in batch 3's slots); batch 2 uses the ACT-sqrt path
        # (legal: its g=3 slot runs after the final Exp op).
        def emit_epilogue_piece(b, g):
            if b < 0:
                return
            if g == 0:
                # DVE-light: the recip chain runs while PE streams the next
                # group, so the B matmuls at g=1 never stall the PE queue
                emit_norm_recip(b)
            elif g == 1:
                for k in range(4):
                    emit_norm_mult(b, k)
                emit_fc_chunk(b, 0)
            elif g == 2:
                emit_fc_chunk(b, 1)
                emit_fc_chunk(b, 2)
            else:
                emit_fc_chunk(b, 3)
                if b == 2:
                    emit_ln_sqrt(2, [0, 1, 2, 3])

        for b in range(BS):
            for g in range(4):
                emit_logits_exp_pair(b, g, 0)
                if b == 0 and g < 2:
                    dummy_avp(3)
                if b < BS - 1 and g == 1:
                    emit_qproj(0, b + 1)
                emit_logits_exp_pair(b, g, 1)
                if b == 0 and g < 2:
                    dummy_avp(3)
                if b < BS - 1 and g == 2:
                    emit_qproj(1, b + 1)
                # AV of the previous group (pipelined one group back)
                if g > 0:
                    emit_av(b, g - 1)
                elif b > 0:
                    emit_av(b - 1, 3)
                    emit_sums_dma(b - 1)
                if b == BS - 1 and g == 3:
                    # last AV pulled forward so the PE reaches it before the
                    # batch-2 epilogue matmuls; sums of groups 0-2 extracted
                    # early so their norm runs during the final AV
                    emit_sums_dma(b, (0, 1, 2))
                    emit_av(b, 3)
                    emit_tail_sums3(b)
                emit_epilogue_piece(b - 1, g)
                if b == BS - 1:
                    if g == 0:
                        emit_rstd01()
                    elif g == 1:
                        emit_apply01(0)
                    elif g == 2:
                        emit_apply01(1)
        # tail: last batch epilogue, ACT-sqrt LN per chunk
        bl = BS - 1
        emit_norm_recip(bl, 0, 6)
        for k in range(3):
            emit_norm_mult(bl, k)
        emit_tail_norm3(bl)
        for sc in range(4):
            emit_fc_chunk(bl, sc)
            emit_ln_sqrt_tail(bl, sc)

    nc.compile()
    return nc


def _prep_core(x, z, fc_b, v):
    """Build the per-core input map (host-side layout packing) for vert v."""
    bf = ml_dtypes.bfloat16
    xv = np.ascontiguousarray(x[:, :, v, :]).reshape(R, D_IN)  # [r, d]
    xT = np.ascontiguousarray(xv.T).astype(bf).reshape(2, 128, R)  # [d, r]
    xres = np.ascontiguousarray(
        (xv + fc_b[None, :]).reshape(16, 128, 256).transpose(1, 0, 2).reshape(128, 16 * 256)
    )
    zv = z.reshape(N_HEAD, NV, BS, SEG, D_Q)[:, v]  # [h, b, t, d]
    # zT carries the logits operand, pre-scaled so logits == log2(exp arg):
    # t = (q . z) * log2e / sqrt(dq)
    zTp = (zv * np.float32(LOG2E * INV_TEMP)).transpose(0, 1, 3, 2)  # [h, b, d, t]
    zT = np.zeros((2, 4, 32, 4, 512), bf)
    for o, h in enumerate(H_PERM):
        for b in range(BS):
            zT[o // 4, o % 4, :, b] = zTp[h, b]
    zT = np.ascontiguousarray(zT.reshape(2, 128, 2048))
    # zA: per av-pair slot q = 4b+g: [headA(g) 33 | headB(g+4) 33]
    zA = np.zeros((4, 128, 66 * 16), bf)
    za_full = np.concatenate(
        [zv, np.ones((N_HEAD, BS, SEG, 1), np.float32)], axis=-1
    ).astype(bf)  # [h, b, t, 33]
    for b in range(BS):
        for g in range(4):
            q = 4 * b + g
            for c in range(4):
                zA[c, :, 66 * q : 66 * q + 33] = za_full[g, b, 128 * c : 128 * (c + 1), :]
                zA[c, :, 66 * q + 33 : 66 * q + 66] = za_full[
                    g + 4, b, 128 * c : 128 * (c + 1), :
                ]
    return {"xT": xT, "xres": xres, "zT": zT, "zA": zA}


def kernel(x, z, w_q_w, w_q_b, fc_w, fc_b, ln_gamma, ln_beta, _trace=False, _tmpdir=None):
    from concourse.bass_utils import run_bass_kernel_spmd

    x = np.asarray(x, np.float32)
    z = np.asarray(z, np.float32)
    w_q_w = np.asarray(w_q_w, np.float32)
    w_q_b = np.asarray(w_q_b, np.float32)
    fc_w = np.asarray(fc_w, np.float32)
    fc_b = np.asarray(fc_b, np.float32)
    ln_gamma = np.asarray(ln_gamma, np.float32)
    ln_beta = np.asarray(ln_beta, np.float32)

    use_wqb = bool(np.any(w_q_b != 0.0))
    use_gb = bool(np.any(ln_gamma != 1.0) or np.any(ln_beta != 0.0))

    key = (use_wqb, use_gb)
    if key not in _prog_cache:
        _prog_cache[key] = _build(use_wqb, use_gb)
    nc = _prog_cache[key]

    bf = ml_dtypes.bfloat16
    # e' permutation: tile T col j -> head H_PERM[4T + j//32], dq j%32
    eperm = np.zeros(256, np.int64)
    for T in range(2):
        for j in range(128):
            o = 4 * T + j // 32
            eperm[128 * T + j] = 32 * H_PERM[o] + j % 32
    wqT = np.ascontiguousarray(w_q_w.T[:, eperm]).astype(bf).reshape(2, 128, 256)
    wqb_p = np.zeros((128, 2), np.float32)
    for T in range(2):
        wqb_p[:, T] = w_q_b[eperm[128 * T : 128 * (T + 1)]]
    # fcT tile k: rows 0:32 = fc rows of head k, rows 64:96 = head k+4, else 0
    fcT_full = fc_w.T  # [e, d_in]
    fcT = np.zeros((4, 128, D_IN), np.float32)
    for k in range(4):
        fcT[k, 0:32] = fcT_full[32 * k : 32 * (k + 1)]
        fcT[k, 64:96] = fcT_full[32 * (k + 4) : 32 * (k + 5)]
    fcT = fcT.astype(bf)
    # ind8 tile: B_k = ind8[:, 128k:128k+128].T @ recb
    ind8 = np.zeros((8, 512), bf)
    for k in range(4):
        ind8[2 * k, 128 * k : 128 * k + 32] = 1.0
        ind8[2 * k + 1, 128 * k + 64 : 128 * k + 96] = 1.0
    shared = {
        "wqT": wqT,
        "wqb": wqb_p,
        "ind8": ind8,
        "fcT": fcT,
        "gbb": np.ascontiguousarray(
            np.concatenate(
                [
                    np.broadcast_to(ln_gamma, (128, 256)),
                    np.broadcast_to(ln_beta, (128, 256)),
                ],
                axis=1,
            )
        ),
    }
    in_maps = []
    for v in range(NCORES):
        m = dict(shared)
        m.update(_prep_core(x, z, fc_b, v))
        in_maps.append(m)

    res = run_bass_kernel_spmd(
        nc,
        in_maps,
        core_ids=list(range(NCORES)),
        trace=_trace,
        tmpdir=_tmpdir,
    )
    out = np.empty((BS, SEG, NV, D_IN), np.float32)
    for v in range(NCORES):
        out[:, :, v, :] = res.results[v]["y"].reshape(BS, SEG, D_IN)
    kernel._last_result = res
    return out


# revision 42
# speedup vs baseline: 1.0014x; 1.0014x over previous
"""Trainium2 Bass kernel for nn_MultiHeadEncDecAttention.

Problem (full shapes):
  x:[4,512,8,256] z:[256,512,32] w_q_w:[256,256] fc_w:[256,256] (+biases, LN params)
  q = x@w_q_w.T (+b) -> [h,v,b,s,dq]; attn = softmax(q@z^T/sqrt(dq)); out = attn@z
  o2 = concat_h(out)@fc_w.T (+b); y = LN(o2 + x)*gamma + beta

Sharding: split on n_verts (nv=8) across the 8 cores - every stage
(q-proj, attention, fc, LN) is independent per vert, so zero cross-core comms.

v3 design (per core, r = b*512+s in [0,2048)):
  Same pipeline skeleton as v2, but the softmax exp - the single biggest
  serial cost (64 ACT ops ~ 90us) - is split across TWO engines:
    - ACT path: table exp, exact. zT is host-prescaled by log2e/sqrt(dq),
      so ACT uses scale=ln2 (exp(t*ln2) = 2^t).
    - DVE path: Schraudolph bit-trick exp in ONE tensor_scalar:
      i16 = rint(128*t + 16254); reinterpret as bf16 == 2^t * (1 +- 3%).
      The +-3% per-weight error cancels in the softmax normalization;
      end-to-end rel err ~ 0.005 (tolerance 2e-2).
  Engine rebalance: qproj/AV casts and sums-cast move ACT-ward (Copy),
  the LN apply moves to GPSIMD, bn_stats/recip/norm-mult stay DVE.
"""

import sys

sys.path.insert(0, "/opt/trn_rl_repo")

from contextlib import ExitStack

import ml_dtypes
import numpy as np

import concourse.bass as bass
import concourse.tile as tile
from concourse import mybir

F32 = mybir.dt.float32
BF16 = mybir.dt.bfloat16
I16 = mybir.dt.int16
AX = mybir.AluOpType
AF = mybir.ActivationFunctionType

N_HEAD = 8
D_Q = 32
D_IN = 256
BS = 4
SEG = 512
NV = 8
LN_EPS = 1e-5
R = BS * SEG  # 2048 rows per core
NCORES = 8
INV_TEMP = 1.0 / np.sqrt(np.float32(D_Q))
LOG2E = 1.4426950408889634
LN2 = 0.6931471805599453
H_PERM = [0, 4, 1, 5, 2, 6, 3, 7]  # head for order index o

# exp engine split: flat unit idx u = (b*8+o)*2+th in [0,64). True -> DVE
# Schraudolph, False -> ACT table exp. ~26/64 on DVE.
DVE_EXP = [u % 8 in (1, 4, 6) or u == 3 for u in range(64)]

_prog_cache = {}


def _build(use_wqb: bool, use_gb: bool):
    from concourse import bacc

    nc = bacc.Bacc("TRN2", target_bir_lowering=False, debug=False)

    d_xT = nc.dram_tensor("xT", [2, 128, R], BF16, kind="ExternalInput").ap()
    d_xres = nc.dram_tensor("xres", [128, 16 * 256], F32, kind="ExternalInput").ap()
    d_zT = nc.dram_tensor("zT", [2, 128, 2048], BF16, kind="ExternalInput").ap()
    d_zA = nc.dram_tensor("zA", [4, 128, 66 * 16], BF16, kind="ExternalInput").ap()
    d_wqT = nc.dram_tensor("wqT", [2, 128, 256], BF16, kind="ExternalInput").ap()
    d_wqb = nc.dram_tensor("wqb", [128, 2], F32, kind="ExternalInput").ap()
    d_fcT = nc.dram_tensor("fcT", [4, 128, D_IN], BF16, kind="ExternalInput").ap()
    d_gbb = nc.dram_tensor("gbb", [128, 512], F32, kind="ExternalInput").ap()
    d_ind8 = nc.dram_tensor("ind8", [8, 512], BF16, kind="ExternalInput").ap()
    d_y = nc.dram_tensor("y", [R, D_IN], F32, kind="ExternalOutput").ap()

    with tile.TileContext(nc) as tc, ExitStack() as ctx:
        P = ctx.enter_context  # noqa

        big = P(tc.tile_pool(name="big", bufs=1))
        lgp = P(tc.tile_pool(name="lgp", bufs=3, space="PSUM"))
        avp = P(tc.tile_pool(name="avp", bufs=2, space="PSUM"))
        expp = P(tc.tile_pool(name="expp", bufs=6))
        smp = P(tc.tile_pool(name="smp", bufs=2))
        stp = P(tc.tile_pool(name="stp", bufs=2))
        outp = P(tc.tile_pool(name="outp", bufs=3))

        # ---- persistent SBUF tiles + input DMAs
        eps_t = big.tile([128, 1], F32)
        nc.vector.memset(eps_t[:], float(LN_EPS))
        dummy_t = big.tile([128, 1], F32)
        # early Exp so the ACT table set loads during the DMA phase
        nc.scalar.activation(dummy_t[:], eps_t[:], AF.Exp)

        # tile for the PE warm-up burst
        warm_t = big.tile([128, 512], BF16, name="warm")
        nc.gpsimd.memset(warm_t[:], 0.0)

        # Inputs are tiled per-batch and DMA'd in first-use order, so batch
        # 0's attention starts after ~0.9 MiB instead of the full ~6 MiB.
        wqT_t = [big.tile([128, 256], BF16, name=f"wqT{k}") for k in range(2)]
        # xT as separate per-chunk tiles so qproj chunk n depends only on its
        # own DMA (dep tracking for DMA writes is whole-tile)
        xT_t = [
            [big.tile([128, 512], BF16, name=f"xT{k}_{n}") for n in range(4)]
            for k in range(2)
        ]
        zT_t = [
            [big.tile([128, 512], BF16, name=f"zT{u}_{b}") for b in range(BS)]
            for u in range(2)
        ]
        zA_t = [
            [big.tile([128, 66 * 4], BF16, name=f"zA{c}_{b}") for b in range(BS)]
            for c in range(4)
        ]
        wqb_t = big.tile([128, 2], F32)
        ind8_t = big.tile([8, 512], BF16)
        fcT_t = [big.tile([128, D_IN], BF16, name=f"fcT{e}") for e in range(4)]
        gbb_t = big.tile([128, 512], F32)
        xres_t = big.tile([128, 16 * 256], F32)

        # gpsimd queue: batch-0 critical path first, then per-batch z data
        for k in range(2):
            nc.gpsimd.dma_start(xT_t[k][0][:], d_xT[k, :, 0:512])
        for u in range(2):
            nc.gpsimd.dma_start(zT_t[u][0][:], d_zT[u, :, 0:512])
        for c in range(4):
            nc.gpsimd.dma_start(zA_t[c][0][:], d_zA[c, :, 0:264])
        for b in range(1, BS):
            for u in range(2):
                nc.gpsimd.dma_start(zT_t[u][b][:], d_zT[u, :, 512 * b : 512 * (b + 1)])
            for c in range(4):
                nc.gpsimd.dma_start(zA_t[c][b][:], d_zA[c, :, 264 * b : 264 * (b + 1)])
        # sync queue: weights, remaining x chunks, residual
        for k in range(2):
            nc.sync.dma_start(wqT_t[k][:], d_wqT[k])
        if use_wqb:
            nc.sync.dma_start(wqb_t[:], d_wqb)
        for k in range(2):
            nc.sync.dma_start(xT_t[k][1][:], d_xT[k, :, 512:1024])
        nc.sync.dma_start(ind8_t[:], d_ind8)
        for e in range(4):
            nc.sync.dma_start(fcT_t[e][:], d_fcT[e])
        for n in range(2, 4):
            for k in range(2):
                nc.sync.dma_start(xT_t[k][n][:], d_xT[k, :, 512 * n : 512 * (n + 1)])
        if use_gb:
            nc.sync.dma_start(gbb_t[:], d_gbb)
        nc.sync.dma_start(xres_t[:], d_xres)

        qT_t = [big.tile([128, R], BF16, name=f"qT{u}") for u in range(2)]
        # outcT tile k: head k rows 0:32, sums row 32; head k+4 rows 64:96,
        # sums row 96; rows 33:63 / 97:127 junk (zeroed by the norm multiply)
        outcT = [big.tile([128, R], BF16, name=f"outcT{e}") for e in range(4)]
        yhold = big.tile([128, 16 * 256], F32)
        mvall = big.tile([128, 32], F32)

        def mm(out, lhsT, rhs, **kw):
            nc.tensor.matmul(out, lhsT, rhs, skip_group_check=True, **kw)

        # zero the never-matmul-written rows of the two av PSUM slots once,
        # so the full-partition cast below never reads non-finite stale PSUM
        for _ in range(2):
            av0 = avp.tile([128, 512], F32, tag="avb", name="av_init")
            nc.vector.memset(av0[32:64, :], 0.0)
            nc.vector.memset(av0[96:128, :], 0.0)

        # PE warm-up burst: ~20 dummy matmuls on junk data with no input
        # deps. They run during the preamble/DMA phase and hold the HAM
        # activity window busy long enough to lock in the fast clock mode
        # (10 was too few - bimodal 108/133us runs; 28 too many).
        for w in range(20):
            wp = lgp.tile([128, 512], F32, tag="lg", name="warmmm")
            mm(wp[:], warm_t[:, 0:128], warm_t[:], start=True, stop=True)

        # ---- q projection chunk: qT[tile T][:, 512n:512n+512]
        def emit_qproj(T, n):
            qp = avp.tile([128, 512], F32, tag="avb", name="qp")
            for k in range(2):
                mm(
                    qp[:],
                    wqT_t[k][:, 128 * T : 128 * (T + 1)],
                    xT_t[k][n][:],
                    start=(k == 0),
                    stop=(k == 1),
                )
            dst = qT_t[T][:, 512 * n : 512 * (n + 1)]
            if use_wqb:
                nc.vector.tensor_scalar(
                    dst, qp[:], wqb_t[:, T : T + 1], 0.0, AX.add, AX.add
                )
            elif n == 0:
                # batch-0 fill phase: DVE is idle, ACT is the exp engine
                nc.vector.tensor_copy(dst, qp[:])
            else:
                nc.scalar.activation(dst, qp[:], AF.Copy)

        # ---- logits + exp for one (order-index o, batch b, t-half th) unit
        expt_tiles = {}

        def _exp_of(b, o, th, lt):
            u = (b * 8 + o) * 2 + th
            expt = expt_tiles[(b, o)]
            dst = expt[:, 1024 * th : 1024 * (th + 1)]
            if DVE_EXP[u]:
                # Schraudolph: i16 = rint(128*t + 16254); bits == bf16 2^t
                nc.vector.tensor_scalar(
                    dst.bitcast(I16), lt[:], 128.0, 16254.0, AX.mult, AX.add
                )
            else:
                nc.scalar.activation(dst, lt[:], AF.Exp, scale=float(LN2))

        def emit_logits_exp_pair(b, g, th):
            # both units (o=2g, 2g+1) of one t-half, logits mms interleaved
            # across the two row-bands so consecutive PE matmuls overlap
            lts = {}
            for o in (2 * g, 2 * g + 1):
                if th == 0:
                    expt_tiles[(b, o)] = expp.tile([128, 2048], BF16, name="expt")
                lts[o] = lgp.tile([128, 1024], F32, tag="lg", name="lt")
            for j in range(2):
                c = 2 * th + j
                for o in (2 * g, 2 * g + 1):
                    T, beta = o // 4, 32 * (o % 4)
                    mm(
                        lts[o][:, 512 * j : 512 * (j + 1)],
                        zT_t[T][b][beta : beta + 32, 128 * c : 128 * (c + 1)],
                        qT_t[T][beta : beta + 32, 512 * b : 512 * (b + 1)],
                        start=True,
                        stop=True,
                        tile_position=(beta, 0),
                    )
            for o in (2 * g, 2 * g + 1):
                _exp_of(b, o, th, lts[o])

        # ---- AV for av-pair g of batch b (heads g and g+4, col-tiled {0,64})
        # split into two emission halves so the matmuls interleave between
        # later logits units (AV of group G-2 never waits on anything)
        av_state = {}

        def emit_av_half(G, half):
            b, g = divmod(G, 4)
            q = 4 * b + g
            if half == 0:
                av_state[G] = (
                    avp.tile([128, 512], F32, tag="avb", name="av"),
                    expt_tiles.pop((b, 2 * g)),
                    expt_tiles.pop((b, 2 * g + 1)),
                )
            av, eA, eB = av_state[G]
            for c in (2 * half, 2 * half + 1):
                mm(
                    av[0:33, :],
                    zA_t[c][b][:, 66 * g : 66 * g + 33],
                    eA[:, 512 * c : 512 * (c + 1)],
                    start=(c == 0),
                    stop=(c == 3),
                )
                mm(
                    av[64:97, :],
                    zA_t[c][b][:, 66 * g + 33 : 66 * g + 66],
                    eB[:, 512 * c : 512 * (c + 1)],
                    start=(c == 0),
                    stop=(c == 3),
                )
            if half == 1:
                dst = outcT[g][:, 512 * b : 512 * (b + 1)]
                if b == 0:
                    nc.vector.tensor_copy(dst, av[:])
                else:
                    nc.scalar.activation(dst, av[:], AF.Copy)
                del av_state[G]

        # ---- epilogue pieces for batch b
        sums_tiles = {}

        def emit_sums_dma(b, ks=(0, 1, 2, 3)):
            if b not in sums_tiles:
                sums_tiles[b] = smp.tile([8, 512], BF16, tag="sums", name="sums_b")
            sums_b = sums_tiles[b]
            for k in ks:
                src = outcT[k][32:97:64, 512 * b : 512 * (b + 1)]
                eng = nc.sync if k % 2 == 0 else nc.gpsimd
                eng.dma_start(sums_b[2 * k : 2 * k + 2, :], src)

        B_tiles = {}
        recip_state = {}

        def emit_norm_recip(b, r0=0, r1=8):
            sums_b = sums_tiles[b]
            if b not in recip_state:
                recip_state[b] = (
                    smp.tile([8, 512], F32, name="sumf"),
                    smp.tile([8, 512], F32, name="recf"),
                    smp.tile([8, 512], BF16, name="recb"),
                )
            sumf, recf, recb = recip_state[b]
            nc.scalar.activation(sumf[r0:r1, :], sums_b[r0:r1, :], AF.Copy)
            nc.vector.reciprocal_approx_fast(recf[r0:r1, :], sumf[r0:r1, :])
            nc.vector.tensor_copy(recb[r0:r1, :], recf[r0:r1, :])
            B_tiles[b] = recb
            if r1 == 8:
                del recip_state[b]
                del sums_tiles[b]

        tail3 = {}

        def emit_tail_sums3(b):
            # k=3 sums into a separate partition-0-aligned [2,512] tile
            sumsB = smp.tile([2, 512], BF16, name="sumsB")
            nc.gpsimd.dma_start(sumsB[:], outcT[3][32:97:64, 512 * b : 512 * (b + 1)])
            tail3["sums"] = sumsB

        def emit_tail_norm3(b):
            sumfB = smp.tile([2, 512], F32, name="sumfB")
            nc.vector.tensor_copy(sumfB[:], tail3["sums"][:])
            recfB = smp.tile([2, 512], F32, name="recfB")
            nc.vector.reciprocal_approx_fast(recfB[:], sumfB[:])
            recbB = smp.tile([2, 512], BF16, name="recbB")
            nc.vector.tensor_copy(recbB[:], recfB[:])
            # ind8's k=0 block has exactly the right row pattern (0:32, 64:96)
            Bt = avp.tile([128, 512], F32, tag="avb", name="Bt3")
            mm(Bt[:], ind8_t[0:2, 0:128], recbB[:], start=True, stop=True)
            sl = outcT[3][:, 512 * b : 512 * (b + 1)]
            nc.vector.tensor_tensor(sl, sl, Bt[:], AX.mult)

        def emit_norm_mult(b, k):
            recb = B_tiles[b]
            Bt = avp.tile([128, 512], F32, tag="avb", name="Bt")
            mm(Bt[:], ind8_t[:, 128 * k : 128 * (k + 1)], recb[:], start=True, stop=True)
            sl = outcT[k][:, 512 * b : 512 * (b + 1)]
            nc.vector.tensor_tensor(sl, sl, Bt[:], AX.mult)

        def emit_fc_chunk(b, sc):
            ci = 4 * b + sc
            reg = avp.tile([128, 512], F32, tag="avb", name="fcp")[:, 0:256]
            for k in range(4):
                mm(
                    reg[:],
                    outcT[k][:, 512 * b + 128 * sc : 512 * b + 128 * (sc + 1)],
                    fcT_t[k][:],
                    start=(k == 0),
                    stop=(k == 3),
                )
            ysl = yhold[:, 256 * ci : 256 * (ci + 1)]
            nc.vector.tensor_tensor(
                ysl, reg[:], xres_t[:, 256 * ci : 256 * (ci + 1)], AX.add
            )
            st6 = stp.tile([128, 6], F32, name="st6")
            nc.vector.bn_stats(st6[:], ysl)
            nc.vector.bn_aggr(mvall[:, 2 * ci : 2 * ci + 2], st6[:])

        def emit_apply(b, sc, y_ap, nmr_ap):
            # yo = (ysl - mu) * rstd, on ACT: Identity(ysl*rstd + (-mu*rstd))
            ci = 4 * b + sc
            ysl = yhold[:, 256 * ci : 256 * (ci + 1)]
            yo = outp.tile([128, 256], F32, name="yo")
            if use_gb:
                t2 = outp.tile([128, 256], F32, tag="t1", name="t2")
                nc.vector.scalar_tensor_tensor(
                    t2[:], ysl, mvall[:, 2 * ci : 2 * ci + 1], gbb_t[:, 0:256],
                    AX.subtract, AX.mult,
                )
                nc.vector.scalar_tensor_tensor(
                    yo[:], t2[:], y_ap, gbb_t[:, 256:512],
                    AX.mult, AX.add,
                )
            else:
                nc.scalar.activation(
                    yo[:], ysl, AF.Identity, bias=nmr_ap, scale=y_ap
                )
            eng = nc.gpsimd if sc % 2 == 0 else nc.sync
            eng.dma_start(d_y[128 * ci : 128 * (ci + 1), :], yo[:])

        def _newton_rstd(va, n):
            y = stp.tile([128, n], F32, tag=f"ny{n}", name="ny")
            nc.vector.reciprocal_approx_fast(y[:], va[:])
            for _ in range(3):
                t1 = stp.tile([128, n], F32, tag=f"nt{n}", name="nt1")
                nc.vector.tensor_tensor(t1[:], y[:], y[:], AX.mult)
                nc.vector.tensor_tensor(t1[:], t1[:], va[:], AX.mult)
                nc.vector.tensor_scalar(t1[:], t1[:], -0.5, 1.5, AX.mult, AX.add)
                nc.vector.tensor_tensor(y[:], y[:], t1[:], AX.mult)
            return y

        def emit_rstd_all(b):
            # rstd = 1/sqrt(var+eps) on DVE (recip-approx seed + 3 Newton
            # rsqrt iterations) - avoids the ACT sqrt table switch entirely.
            mvb = mvall[:, 8 * b : 8 * (b + 1)].rearrange("p (c two) -> p c two", two=2)
            va = stp.tile([128, 4], F32, tag="va4", name="va")
            nc.vector.tensor_scalar(va[:], mvb[:, :, 1:2], eps_t[:], 0.0, AX.add, AX.add)
            y = _newton_rstd(va, 4)
            # nmr = (-mu) * rstd (per-chunk bias for the ACT apply)
            nmr = stp.tile([128, 4], F32, tag="nm4", name="nmr")
            nc.vector.scalar_tensor_tensor(
                nmr[:], mvb[:, :, 0:1], -1.0, y[:], AX.mult, AX.mult
            )
            return y, nmr

        def emit_rstd_sqrt(b, cols):
            # tail-only rstd: ACT Sqrt(var+eps) (sqrt table set - only legal
            # after the LAST Exp op) + DVE fast reciprocal. ~4 ops total vs
            # a ~13-op Newton chain.
            n = len(cols)
            c0 = cols[0]
            assert cols == list(range(c0, c0 + n))
            mvb = mvall[:, 8 * b : 8 * (b + 1)].rearrange("p (c two) -> p c two", two=2)
            var_ap = mvb[:, c0 : c0 + n, 1:2]
            mu_ap = mvb[:, c0 : c0 + n, 0:1]
            sqv = stp.tile([128, n], F32, tag=f"sq{n}", name="sqv")
            nc.scalar.activation(sqv[:], var_ap, AF.Sqrt, bias=eps_t[:, 0:1])
            y = stp.tile([128, n], F32, tag=f"sy{n}", name="sy")
            nc.vector.reciprocal_approx_fast(y[:], sqv[:])
            nmr = stp.tile([128, n], F32, tag=f"sm{n}", name="smr")
            nc.vector.scalar_tensor_tensor(
                nmr[:], mu_ap, -1.0, y[:], AX.mult, AX.mult
            )
            return y, nmr

        def emit_ln_sqrt(b, cols):
            y, nmr = emit_rstd_sqrt(b, cols)
            for j, sc in enumerate(cols):
                emit_apply(b, sc, y[:, j : j + 1], nmr[:, j : j + 1])

        def emit_ln_sqrt_tail(b, sc):
            # tail chunk: sqrt -> fast recip -> DVE apply (shortest chain to
            # the final output DMA; DVE is idle in the tail)
            if use_gb:
                emit_ln_sqrt(b, [sc])
                return
            ci = 4 * b + sc
            sqv = stp.tile([128, 1], F32, tag="sq1", name="sqv1")
            nc.scalar.activation(
                sqv[:], mvall[:, 2 * ci + 1 : 2 * ci + 2], AF.Sqrt,
                bias=eps_t[:, 0:1],
            )
            y = stp.tile([128, 1], F32, tag="sy1", name="sy1")
            nc.vector.reciprocal_approx_fast(y[:], sqv[:])
            ysl = yhold[:, 256 * ci : 256 * (ci + 1)]
            yo = outp.tile([128, 256], F32, name="yo")
            nc.vector.tensor_scalar(
                yo[:], ysl, mvall[:, 2 * ci : 2 * ci + 1], y[:, 0:1],
                AX.subtract, AX.mult,
            )
            eng = nc.gpsimd if sc % 2 == 0 else nc.sync
            eng.dma_start(d_y[128 * ci : 128 * (ci + 1), :], yo[:])

        def emit_ln_finish(b):
            y, nmr = emit_rstd_all(b)
            for sc in range(4):
                emit_apply(b, sc, y[:, sc : sc + 1], nmr[:, sc : sc + 1])

        # one merged Newton rstd chain for batches 0 and 1 ([128, 8]),
        # emitted once batch 1's bn stats are all aggregated
        chain01 = {}

        def emit_rstd01():
            mvb = mvall[:, 0:16].rearrange("p (c two) -> p c two", two=2)
            va = stp.tile([128, 8], F32, tag="va8", name="va8")
            nc.vector.tensor_scalar(va[:], mvb[:, :, 1:2], eps_t[:], 0.0, AX.add, AX.add)
            y = _newton_rstd(va, 8)
            nmr = stp.tile([128, 8], F32, tag="nm8", name="nmr8")
            nc.vector.scalar_tensor_tensor(
                nmr[:], mvb[:, :, 0:1], -1.0, y[:], AX.mult, AX.mult
            )
            chain01["y"] = y
            chain01["nmr"] = nmr

        def emit_apply01(b):
            y, nmr = chain01["y"], chain01["nmr"]
            for sc in range(4):
                j = 4 * b + sc
                emit_apply(b, sc, y[:, j : j + 1], nmr[:, j : j + 1])

        # ---- main schedule -------------------------------------------------
        # qproj for batch 0 first (only needs wqT + xT chunk 0)
        emit_qproj(0, 0)
        emit_qproj(1, 0)
        # bridge dummies: keep the PE busy between qproj and the first logits
        for w in range(8):
            wp = lgp.tile([128, 512], F32, tag="lg", name="bridge")
            mm(wp[:], warm_t[:, 0:128], warm_t[:], start=True, stop=True)

        def emit_av(b, g):
            emit_av_half(4 * b + g, 0)
            emit_av_half(4 * b + g, 1)

        # epilogue pieces of batch b-1, spread across batch b's groups.
        # LN applies for batches 0/1 are deferred to the merged chain01
        # (emitted in batch 3's slots); batch 2 uses the ACT-sqrt path
        # (legal: its g=3 slot runs after the final Exp op).
        def emit_epilogue_piece(b, g):
            if b < 0:
                return
            if g == 0:
                # DVE-light: the recip chain runs while PE streams the next
                # group, so the B matmuls at g=1 never stall the PE queue
                emit_norm_recip(b)
            elif g == 1:
                for k in range(4):
                    emit_norm_mult(b, k)
                emit_fc_chunk(b, 0)
            elif g == 2:
                emit_fc_chunk(b, 1)
                emit_fc_chunk(b, 2)
            else:
                emit_fc_chunk(b, 3)
                if b == 2:
                    emit_ln_sqrt(2, [0, 1, 2, 3])

        for b in range(BS):
            for g in range(4):
                emit_logits_exp_pair(b, g, 0)
                if b < BS - 1 and g == 1:
                    emit_qproj(0, b + 1)
                emit_logits_exp_pair(b, g, 1)
                if b < BS - 1 and g == 2:
                    emit_qproj(1, b + 1)
                # AV of the previous group (pipelined one group back)
                if g > 0:
                    emit_av(b, g - 1)
                elif b > 0:
                    emit_av(b - 1, 3)
                    emit_sums_dma(b - 1)
                if b == BS - 1 and g == 3:
                    # last AV pulled forward so the PE reaches it before the
                    # batch-2 epilogue matmuls; sums of groups 0-2 extracted
                    # early so their norm runs during the final AV
                    emit_sums_dma(b, (0, 1, 2))
                    emit_av(b, 3)
                    emit_tail_sums3(b)
                emit_epilogue_piece(b - 1, g)
                if b == BS - 1:
                    if g == 0:
                        emit_rstd01()
                    elif g == 1:
                        emit_apply01(0)
                    elif g == 2:
                        emit_apply01(1)
        # tail: last batch epilogue, ACT-sqrt LN per chunk
        bl = BS - 1
        emit_norm_recip(bl, 0, 6)
        for k in range(3):
            emit_norm_mult(bl, k)
        emit_tail_norm3(bl)
        for sc in range(4):
            emit_fc_chunk(bl, sc)
            emit_ln_sqrt_tail(bl, sc)

    nc.compile()
    return nc


def _prep_core(x, z, fc_b, v):
    """Build the per-core input map (host-side layout packing) for vert v."""
    bf = ml_dtypes.bfloat16
    xv = np.ascontiguousarray(x[:, :, v, :]).reshape(R, D_IN)  # [r, d]
    xT = np.ascontiguousarray(xv.T).astype(bf).reshape(2, 128, R)  # [d, r]
    xres = np.ascontiguousarray(
        (xv + fc_b[None, :]).reshape(16, 128, 256).transpose(1, 0, 2).reshape(128, 16 * 256)
    )
    zv = z.reshape(N_HEAD, NV, BS, SEG, D_Q)[:, v]  # [h, b, t, d]
    # zT carries the logits operand, pre-scaled so logits == log2(exp arg):
    # t = (q . z) * log2e / sqrt(dq)
    zTp = (zv * np.float32(LOG2E * INV_TEMP)).transpose(0, 1, 3, 2)  # [h, b, d, t]
    zT = np.zeros((2, 4, 32, 4, 512), bf)
    for o, h in enumerate(H_PERM):
        for b in range(BS):
            zT[o // 4, o % 4, :, b] = zTp[h, b]
    zT = np.ascontiguousarray(zT.reshape(2, 128, 2048))
    # zA: per av-pair slot q = 4b+g: [headA(g) 33 | headB(g+4) 33]
    zA = np.zeros((4, 128, 66 * 16), bf)
    za_full = np.concatenate(
        [zv, np.ones((N_HEAD, BS, SEG, 1), np.float32)], axis=-1
    ).astype(bf)  # [h, b, t, 33]
    for b in range(BS):
        for g in range(4):
            q = 4 * b + g
            for c in range(4):
                zA[c, :, 66 * q : 66 * q + 33] = za_full[g, b, 128 * c : 128 * (c + 1), :]
                zA[c, :, 66 * q + 33 : 66 * q + 66] = za_full[
                    g + 4, b, 128 * c : 128 * (c + 1), :
                ]
    return {"xT": xT, "xres": xres, "zT": zT, "zA": zA}


def kernel(x, z, w_q_w, w_q_b, fc_w, fc_b, ln_gamma, ln_beta, _trace=False, _tmpdir=None):
    from concourse.bass_utils import run_bass_kernel_spmd

    x = np.asarray(x, np.float32)
    z = np.asarray(z, np.float32)
    w_q_w = np.asarray(w_q_w, np.float32)
    w_q_b = np.asarray(w_q_b, np.float32)
    fc_w = np.asarray(fc_w, np.float32)
    fc_b = np.asarray(fc_b, np.float32)
    ln_gamma = np.asarray(ln_gamma, np.float32)
    ln_beta = np.asarray(ln_beta, np.float32)

    use_wqb = bool(np.any(w_q_b != 0.0))
    use_gb = bool(np.any(ln_gamma != 1.0) or np.any(ln_beta != 0.0))

    key = (use_wqb, use_gb)
    if key not in _prog_cache:
        _prog_cache[key] = _build(use_wqb, use_gb)
    nc = _prog_cache[key]

    bf = ml_dtypes.bfloat16
    # e' permutation: tile T col j -> head H_PERM[4T + j//32], dq j%32
    eperm = np.zeros(256, np.int64)
    for T in range(2):
        for j in range(128):
            o = 4 * T + j // 32
            eperm[128 * T + j] = 32 * H_PERM[o] + j % 32
    wqT = np.ascontiguousarray(w_q_w.T[:, eperm]).astype(bf).reshape(2, 128, 256)
    wqb_p = np.zeros((128, 2), np.float32)
    for T in range(2):
        wqb_p[:, T] = w_q_b[eperm[128 * T : 128 * (T + 1)]]
    # fcT tile k: rows 0:32 = fc rows of head k, rows 64:96 = head k+4, else 0
    fcT_full = fc_w.T  # [e, d_in]
    fcT = np.zeros((4, 128, D_IN), np.float32)
    for k in range(4):
        fcT[k, 0:32] = fcT_full[32 * k : 32 * (k + 1)]
        fcT[k, 64:96] = fcT_full[32 * (k + 4) : 32 * (k + 5)]
    fcT = fcT.astype(bf)
    # ind8 tile: B_k = ind8[:, 128k:128k+128].T @ recb
    ind8 = np.zeros((8, 512), bf)
    for k in range(4):
        ind8[2 * k, 128 * k : 128 * k + 32] = 1.0
        ind8[2 * k + 1, 128 * k + 64 : 128 * k + 96] = 1.0
    shared = {
        "wqT": wqT,
        "wqb": wqb_p,
        "ind8": ind8,
        "fcT": fcT,
        "gbb": np.ascontiguousarray(
            np.concatenate(
                [
                    np.broadcast_to(ln_gamma, (128, 256)),
                    np.broadcast_to(ln_beta, (128, 256)),
                ],
                axis=1,
            )
        ),
    }
    in_maps = []
    for v in range(NCORES):
        m = dict(shared)
        m.update(_prep_core(x, z, fc_b, v))
        in_maps.append(m)

    res = run_bass_kernel_spmd(
        nc,
        in_maps,
        core_ids=list(range(NCORES)),
        trace=_trace,
        tmpdir=_tmpdir,
    )
    out = np.empty((BS, SEG, NV, D_IN), np.float32)
    for v in range(NCORES):
        out[:, :, v, :] = res.results[v]["y"].reshape(BS, SEG, D_IN)
    kernel._last_result = res
    return out


# revision 44
# speedup vs baseline: 1.0128x; 1.0114x over previous
"""Trainium2 Bass kernel for nn_MultiHeadEncDecAttention.

Problem (full shapes):
  x:[4,512,8,256] z:[256,512,32] w_q_w:[256,256] fc_w:[256,256] (+biases, LN params)
  q = x@w_q_w.T (+b) -> [h,v,b,s,dq]; attn = softmax(q@z^T/sqrt(dq)); out = attn@z
  o2 = concat_h(out)@fc_w.T (+b); y = LN(o2 + x)*gamma + beta

Sharding: split on n_verts (nv=8) across the 8 cores - every stage
(q-proj, attention, fc, LN) is independent per vert, so zero cross-core comms.

v3 design (per core, r = b*512+s in [0,2048)):
  Same pipeline skeleton as v2, but the softmax exp - the single biggest
  serial cost (64 ACT ops ~ 90us) - is split across TWO engines:
    - ACT path: table exp, exact. zT is host-prescaled by log2e/sqrt(dq),
      so ACT uses scale=ln2 (exp(t*ln2) = 2^t).
    - DVE path: Schraudolph bit-trick exp in ONE tensor_scalar:
      i16 = rint(128*t + 16254); reinterpret as bf16 == 2^t * (1 +- 3%).
      The +-3% per-weight error cancels in the softmax normalization;
      end-to-end rel err ~ 0.005 (tolerance 2e-2).
  Engine rebalance: qproj/AV casts and sums-cast move ACT-ward (Copy),
  the LN apply moves to GPSIMD, bn_stats/recip/norm-mult stay DVE.
"""

import sys

sys.path.insert(0, "/opt/trn_rl_repo")

from contextlib import ExitStack

import ml_dtypes
import numpy as np

import concourse.bass as bass
import concourse.tile as tile
from concourse import mybir

F32 = mybir.dt.float32
BF16 = mybir.dt.bfloat16
I16 = mybir.dt.int16
AX = mybir.AluOpType
AF = mybir.ActivationFunctionType

N_HEAD = 8
D_Q = 32
D_IN = 256
BS = 4
SEG = 512
NV = 8
LN_EPS = 1e-5
R = BS * SEG  # 2048 rows per core
NCORES = 8
INV_TEMP = 1.0 / np.sqrt(np.float32(D_Q))
LOG2E = 1.4426950408889634
LN2 = 0.6931471805599453
H_PERM = [0, 4, 1, 5, 2, 6, 3, 7]  # head for order index o

# exp engine split: flat unit idx u = (b*8+o)*2+th in [0,64). True -> DVE
# Schraudolph, False -> ACT table exp. ~26/64 on DVE.
DVE_EXP = [u % 8 in (1, 4, 6) or u == 3 for u in range(64)]

_prog_cache = {}


def _build(use_wqb: bool, use_gb: bool):
    from concourse import bacc

    nc = bacc.Bacc("TRN2", target_bir_lowering=False, debug=False)

    d_xT = nc.dram_tensor("xT", [2, 128, R], BF16, kind="ExternalInput").ap()
    d_xres = nc.dram_tensor("xres", [128, 16 * 256], F32, kind="ExternalInput").ap()
    d_zT = nc.dram_tensor("zT", [2, 128, 2048], BF16, kind="ExternalInput").ap()
    d_zA = nc.dram_tensor("zA", [4, 128, 66 * 16], BF16, kind="ExternalInput").ap()
    d_wqT = nc.dram_tensor("wqT", [2, 128, 256], BF16, kind="ExternalInput").ap()
    d_wqb = nc.dram_tensor("wqb", [128, 2], F32, kind="ExternalInput").ap()
    d_fcT = nc.dram_tensor("fcT", [4, 128, D_IN], BF16, kind="ExternalInput").ap()
    d_gbb = nc.dram_tensor("gbb", [128, 512], F32, kind="ExternalInput").ap()
    d_ind8 = nc.dram_tensor("ind8", [8, 512], BF16, kind="ExternalInput").ap()
    d_y = nc.dram_tensor("y", [R, D_IN], F32, kind="ExternalOutput").ap()

    with tile.TileContext(nc) as tc, ExitStack() as ctx:
        P = ctx.enter_context  # noqa

        big = P(tc.tile_pool(name="big", bufs=1))
        lgp = P(tc.tile_pool(name="lgp", bufs=3, space="PSUM"))
        avp = P(tc.tile_pool(name="avp", bufs=2, space="PSUM"))
        expp = P(tc.tile_pool(name="expp", bufs=6))
        smp = P(tc.tile_pool(name="smp", bufs=2))
        stp = P(tc.tile_pool(name="stp", bufs=2))
        outp = P(tc.tile_pool(name="outp", bufs=3))

        # ---- persistent SBUF tiles + input DMAs
        eps_t = big.tile([128, 1], F32)
        nc.vector.memset(eps_t[:], float(LN_EPS))
        dummy_t = big.tile([128, 1], F32)
        # early Exp so the ACT table set loads during the DMA phase
        nc.scalar.activation(dummy_t[:], eps_t[:], AF.Exp)

        # tile for the PE warm-up burst
        warm_t = big.tile([128, 512], BF16, name="warm")
        nc.gpsimd.memset(warm_t[:], 0.0)

        # Inputs are tiled per-batch and DMA'd in first-use order, so batch
        # 0's attention starts after ~0.9 MiB instead of the full ~6 MiB.
        wqT_t = [big.tile([128, 256], BF16, name=f"wqT{k}") for k in range(2)]
        # xT as separate per-chunk tiles so qproj chunk n depends only on its
        # own DMA (dep tracking for DMA writes is whole-tile)
        xT_t = [
            [big.tile([128, 512], BF16, name=f"xT{k}_{n}") for n in range(4)]
            for k in range(2)
        ]
        zT_t = [
            [big.tile([128, 512], BF16, name=f"zT{u}_{b}") for b in range(BS)]
            for u in range(2)
        ]
        zA_t = [
            [big.tile([128, 66 * 4], BF16, name=f"zA{c}_{b}") for b in range(BS)]
            for c in range(4)
        ]
        wqb_t = big.tile([128, 2], F32)
        ind8_t = big.tile([8, 512], BF16)
        fcT_t = [big.tile([128, D_IN], BF16, name=f"fcT{e}") for e in range(4)]
        gbb_t = big.tile([128, 512], F32)
        xres_t = big.tile([128, 16 * 256], F32)

        # gpsimd queue: batch-0 critical path first, then per-batch z data
        for k in range(2):
            nc.gpsimd.dma_start(xT_t[k][0][:], d_xT[k, :, 0:512])
        for u in range(2):
            nc.gpsimd.dma_start(zT_t[u][0][:], d_zT[u, :, 0:512])
        for c in range(4):
            nc.gpsimd.dma_start(zA_t[c][0][:], d_zA[c, :, 0:264])
        for b in range(1, BS):
            for u in range(2):
                nc.gpsimd.dma_start(zT_t[u][b][:], d_zT[u, :, 512 * b : 512 * (b + 1)])
            for c in range(4):
                nc.gpsimd.dma_start(zA_t[c][b][:], d_zA[c, :, 264 * b : 264 * (b + 1)])
        # sync queue: weights, remaining x chunks, residual
        for k in range(2):
            nc.sync.dma_start(wqT_t[k][:], d_wqT[k])
        if use_wqb:
            nc.sync.dma_start(wqb_t[:], d_wqb)
        for k in range(2):
            nc.sync.dma_start(xT_t[k][1][:], d_xT[k, :, 512:1024])
        nc.sync.dma_start(ind8_t[:], d_ind8)
        for e in range(4):
            nc.sync.dma_start(fcT_t[e][:], d_fcT[e])
        for n in range(2, 4):
            for k in range(2):
                nc.sync.dma_start(xT_t[k][n][:], d_xT[k, :, 512 * n : 512 * (n + 1)])
        if use_gb:
            nc.sync.dma_start(gbb_t[:], d_gbb)
        nc.sync.dma_start(xres_t[:], d_xres)

        qT_t = [big.tile([128, R], BF16, name=f"qT{u}") for u in range(2)]
        # outcT tile k: head k rows 0:32, sums row 32; head k+4 rows 64:96,
        # sums row 96; rows 33:63 / 97:127 junk (zeroed by the norm multiply)
        outcT = [big.tile([128, R], BF16, name=f"outcT{e}") for e in range(4)]
        yhold = big.tile([128, 16 * 256], F32)
        mvall = big.tile([128, 32], F32)

        def mm(out, lhsT, rhs, **kw):
            nc.tensor.matmul(out, lhsT, rhs, skip_group_check=True, **kw)

        # zero the never-matmul-written rows of the two av PSUM slots once,
        # so the full-partition cast below never reads non-finite stale PSUM
        for _ in range(2):
            av0 = avp.tile([128, 512], F32, tag="avb", name="av_init")
            nc.vector.memset(av0[32:64, :], 0.0)
            nc.vector.memset(av0[96:128, :], 0.0)

        # PE warm-up burst: ~20 dummy matmuls on junk data with no input
        # deps. They run during the preamble/DMA phase and hold the HAM
        # activity window busy long enough to lock in the fast clock mode
        # (10 was too few - bimodal 108/133us runs; 28 too many).
        for w in range(20):
            wp = lgp.tile([128, 512], F32, tag="lg", name="warmmm")
            mm(wp[:], warm_t[:, 0:128], warm_t[:], start=True, stop=True)

        # ---- q projection chunk: qT[tile T][:, 512n:512n+512]
        def emit_qproj(T, n):
            qp = avp.tile([128, 512], F32, tag="avb", name="qp")
            for k in range(2):
                mm(
                    qp[:],
                    wqT_t[k][:, 128 * T : 128 * (T + 1)],
                    xT_t[k][n][:],
                    start=(k == 0),
                    stop=(k == 1),
                )
            dst = qT_t[T][:, 512 * n : 512 * (n + 1)]
            if use_wqb:
                nc.vector.tensor_scalar(
                    dst, qp[:], wqb_t[:, T : T + 1], 0.0, AX.add, AX.add
                )
            elif n == 0:
                # batch-0 fill phase: DVE is idle, ACT is the exp engine
                nc.vector.tensor_copy(dst, qp[:])
            else:
                nc.scalar.activation(dst, qp[:], AF.Copy)

        # ---- logits + exp for one (order-index o, batch b, t-half th) unit
        expt_tiles = {}

        def _exp_of(b, o, th, lt):
            u = (b * 8 + o) * 2 + th
            expt = expt_tiles[(b, o)]
            dst = expt[:, 1024 * th : 1024 * (th + 1)]
            if DVE_EXP[u]:
                # Schraudolph: i16 = rint(128*t + 16254); bits == bf16 2^t
                nc.vector.tensor_scalar(
                    dst.bitcast(I16), lt[:], 128.0, 16254.0, AX.mult, AX.add
                )
            else:
                nc.scalar.activation(dst, lt[:], AF.Exp, scale=float(LN2))

        def emit_logits_exp_pair(b, g, th):
            # both units (o=2g, 2g+1) of one t-half, logits mms interleaved
            # across the two row-bands so consecutive PE matmuls overlap
            lts = {}
            for o in (2 * g, 2 * g + 1):
                if th == 0:
                    expt_tiles[(b, o)] = expp.tile([128, 2048], BF16, name="expt")
                lts[o] = lgp.tile([128, 1024], F32, tag="lg", name="lt")
            for j in range(2):
                c = 2 * th + j
                for o in (2 * g, 2 * g + 1):
                    T, beta = o // 4, 32 * (o % 4)
                    mm(
                        lts[o][:, 512 * j : 512 * (j + 1)],
                        zT_t[T][b][beta : beta + 32, 128 * c : 128 * (c + 1)],
                        qT_t[T][beta : beta + 32, 512 * b : 512 * (b + 1)],
                        start=True,
                        stop=True,
                        tile_position=(beta, 0),
                    )
            for o in (2 * g, 2 * g + 1):
                _exp_of(b, o, th, lts[o])

        # ---- AV for av-pair g of batch b (heads g and g+4, col-tiled {0,64})
        # split into two emission halves so the matmuls interleave between
        # later logits units (AV of group G-2 never waits on anything)
        av_state = {}

        def emit_av_half(G, half):
            b, g = divmod(G, 4)
            q = 4 * b + g
            if half == 0:
                av_state[G] = (
                    avp.tile([128, 512], F32, tag="avb", name="av"),
                    expt_tiles.pop((b, 2 * g)),
                    expt_tiles.pop((b, 2 * g + 1)),
                )
            av, eA, eB = av_state[G]
            for c in (2 * half, 2 * half + 1):
                mm(
                    av[0:33, :],
                    zA_t[c][b][:, 66 * g : 66 * g + 33],
                    eA[:, 512 * c : 512 * (c + 1)],
                    start=(c == 0),
                    stop=(c == 3),
                )
                mm(
                    av[64:97, :],
                    zA_t[c][b][:, 66 * g + 33 : 66 * g + 66],
                    eB[:, 512 * c : 512 * (c + 1)],
                    start=(c == 0),
                    stop=(c == 3),
                )
            if half == 1:
                dst = outcT[g][:, 512 * b : 512 * (b + 1)]
                if b == 0:
                    nc.vector.tensor_copy(dst, av[:])
                else:
                    nc.scalar.activation(dst, av[:], AF.Copy)
                del av_state[G]

        # ---- epilogue pieces for batch b
        sums_tiles = {}

        def emit_sums_dma(b, ks=(0, 1, 2, 3)):
            if b not in sums_tiles:
                sums_tiles[b] = smp.tile([8, 512], BF16, tag="sums", name="sums_b")
            sums_b = sums_tiles[b]
            for k in ks:
                src = outcT[k][32:97:64, 512 * b : 512 * (b + 1)]
                eng = nc.sync if k % 2 == 0 else nc.gpsimd
                eng.dma_start(sums_b[2 * k : 2 * k + 2, :], src)

        B_tiles = {}
        recip_state = {}

        def emit_norm_recip(b, r0=0, r1=8):
            sums_b = sums_tiles[b]
            if b not in recip_state:
                recip_state[b] = (
                    smp.tile([8, 512], F32, name="sumf"),
                    smp.tile([8, 512], F32, name="recf"),
                    smp.tile([8, 512], BF16, name="recb"),
                )
            sumf, recf, recb = recip_state[b]
            nc.scalar.activation(sumf[r0:r1, :], sums_b[r0:r1, :], AF.Copy)
            nc.vector.reciprocal_approx_fast(recf[r0:r1, :], sumf[r0:r1, :])
            nc.vector.tensor_copy(recb[r0:r1, :], recf[r0:r1, :])
            B_tiles[b] = recb
            if r1 == 8:
                del recip_state[b]
                del sums_tiles[b]

        tail3 = {}

        def emit_tail_sums3(b):
            # k=3 sums into a separate partition-0-aligned [2,512] tile
            sumsB = smp.tile([2, 512], BF16, name="sumsB")
            nc.gpsimd.dma_start(sumsB[:], outcT[3][32:97:64, 512 * b : 512 * (b + 1)])
            tail3["sums"] = sumsB

        def emit_tail_norm3(b):
            sumfB = smp.tile([2, 512], F32, name="sumfB")
            nc.vector.tensor_copy(sumfB[:], tail3["sums"][:])
            recfB = smp.tile([2, 512], F32, name="recfB")
            nc.vector.reciprocal_approx_fast(recfB[:], sumfB[:])
            recbB = smp.tile([2, 512], BF16, name="recbB")
            nc.vector.tensor_copy(recbB[:], recfB[:])
            # ind8's k=0 block has exactly the right row pattern (0:32, 64:96)
            Bt = avp.tile([128, 512], F32, tag="avb", name="Bt3")
            mm(Bt[:], ind8_t[0:2, 0:128], recbB[:], start=True, stop=True)
            sl = outcT[3][:, 512 * b : 512 * (b + 1)]
            nc.vector.tensor_tensor(sl, sl, Bt[:], AX.mult)

        def emit_norm_mult(b, k):
            recb = B_tiles[b]
            Bt = avp.tile([128, 512], F32, tag="avb", name="Bt")
            mm(Bt[:], ind8_t[:, 128 * k : 128 * (k + 1)], recb[:], start=True, stop=True)
            sl = outcT[k][:, 512 * b : 512 * (b + 1)]
            nc.vector.tensor_tensor(sl, sl, Bt[:], AX.mult)

        def emit_fc_chunk(b, sc):
            ci = 4 * b + sc
            reg = avp.tile([128, 512], F32, tag="avb", name="fcp")[:, 0:256]
            for k in range(4):
                mm(
                    reg[:],
                    outcT[k][:, 512 * b + 128 * sc : 512 * b + 128 * (sc + 1)],
                    fcT_t[k][:],
                    start=(k == 0),
                    stop=(k == 3),
                )
            ysl = yhold[:, 256 * ci : 256 * (ci + 1)]
            nc.vector.tensor_tensor(
                ysl, reg[:], xres_t[:, 256 * ci : 256 * (ci + 1)], AX.add
            )
            st6 = stp.tile([128, 6], F32, name="st6")
            nc.vector.bn_stats(st6[:], ysl)
            nc.vector.bn_aggr(mvall[:, 2 * ci : 2 * ci + 2], st6[:])

        def emit_apply(b, sc, y_ap, nmr_ap):
            # yo = (ysl - mu) * rstd, on ACT: Identity(ysl*rstd + (-mu*rstd))
            ci = 4 * b + sc
            ysl = yhold[:, 256 * ci : 256 * (ci + 1)]
            yo = outp.tile([128, 256], F32, name="yo")
            if use_gb:
                t2 = outp.tile([128, 256], F32, tag="t1", name="t2")
                nc.vector.scalar_tensor_tensor(
                    t2[:], ysl, mvall[:, 2 * ci : 2 * ci + 1], gbb_t[:, 0:256],
                    AX.subtract, AX.mult,
                )
                nc.vector.scalar_tensor_tensor(
                    yo[:], t2[:], y_ap, gbb_t[:, 256:512],
                    AX.mult, AX.add,
                )
            else:
                nc.scalar.activation(
                    yo[:], ysl, AF.Identity, bias=nmr_ap, scale=y_ap
                )
            eng = nc.gpsimd if sc % 2 == 0 else nc.sync
            eng.dma_start(d_y[128 * ci : 128 * (ci + 1), :], yo[:])

        def _newton_rstd(va, n):
            y = stp.tile([128, n], F32, tag=f"ny{n}", name="ny")
            nc.vector.reciprocal_approx_fast(y[:], va[:])
            for _ in range(3):
                t1 = stp.tile([128, n], F32, tag=f"nt{n}", name="nt1")
                nc.vector.tensor_tensor(t1[:], y[:], y[:], AX.mult)
                nc.vector.tensor_tensor(t1[:], t1[:], va[:], AX.mult)
                nc.vector.tensor_scalar(t1[:], t1[:], -0.5, 1.5, AX.mult, AX.add)
                nc.vector.tensor_tensor(y[:], y[:], t1[:], AX.mult)
            return y

        def emit_rstd_all(b):
            # rstd = 1/sqrt(var+eps) on DVE (recip-approx seed + 3 Newton
            # rsqrt iterations) - avoids the ACT sqrt table switch entirely.
            mvb = mvall[:, 8 * b : 8 * (b + 1)].rearrange("p (c two) -> p c two", two=2)
            va = stp.tile([128, 4], F32, tag="va4", name="va")
            nc.vector.tensor_scalar(va[:], mvb[:, :, 1:2], eps_t[:], 0.0, AX.add, AX.add)
            y = _newton_rstd(va, 4)
            # nmr = (-mu) * rstd (per-chunk bias for the ACT apply)
            nmr = stp.tile([128, 4], F32, tag="nm4", name="nmr")
            nc.vector.scalar_tensor_tensor(
                nmr[:], mvb[:, :, 0:1], -1.0, y[:], AX.mult, AX.mult
            )
            return y, nmr

        def emit_rstd_sqrt(b, cols):
            # tail-only rstd: ACT Sqrt(var+eps) (sqrt table set - only legal
            # after the LAST Exp op) + DVE fast reciprocal. ~4 ops total vs
            # a ~13-op Newton chain.
            n = len(cols)
            c0 = cols[0]
            assert cols == list(range(c0, c0 + n))
            mvb = mvall[:, 8 * b : 8 * (b + 1)].rearrange("p (c two) -> p c two", two=2)
            var_ap = mvb[:, c0 : c0 + n, 1:2]
            mu_ap = mvb[:, c0 : c0 + n, 0:1]
            sqv = stp.tile([128, n], F32, tag=f"sq{n}", name="sqv")
            nc.scalar.activation(sqv[:], var_ap, AF.Sqrt, bias=eps_t[:, 0:1])
            y = stp.tile([128, n], F32, tag=f"sy{n}", name="sy")
            nc.vector.reciprocal_approx_fast(y[:], sqv[:])
            nmr = stp.tile([128, n], F32, tag=f"sm{n}", name="smr")
            nc.vector.scalar_tensor_tensor(
                nmr[:], mu_ap, -1.0, y[:], AX.mult, AX.mult
            )
            return y, nmr

        def emit_ln_sqrt(b, cols):
            y, nmr = emit_rstd_sqrt(b, cols)
            for j, sc in enumerate(cols):
                emit_apply(b, sc, y[:, j : j + 1], nmr[:, j : j + 1])

        def emit_ln_sqrt_tail(b, sc):
            # tail chunk: sqrt -> fast recip -> DVE apply (shortest chain to
            # the final output DMA; DVE is idle in the tail)
            if use_gb:
                emit_ln_sqrt(b, [sc])
                return
            ci = 4 * b + sc
            sqv = stp.tile([128, 1], F32, tag="sq1", name="sqv1")
            nc.scalar.activation(
                sqv[:], mvall[:, 2 * ci + 1 : 2 * ci + 2], AF.Sqrt,
                bias=eps_t[:, 0:1],
            )
            y = stp.tile([128, 1], F32, tag="sy1", name="sy1")
            nc.vector.reciprocal_approx_fast(y[:], sqv[:])
            ysl = yhold[:, 256 * ci : 256 * (ci + 1)]
            yo = outp.tile([128, 256], F32, name="yo")
            nc.vector.tensor_scalar(
                yo[:], ysl, mvall[:, 2 * ci : 2 * ci + 1], y[:, 0:1],
                AX.subtract, AX.mult,
            )
            eng = nc.gpsimd if sc % 2 == 0 else nc.sync
            eng.dma_start(d_y[128 * ci : 128 * (ci + 1), :], yo[:])

        def emit_ln_finish(b):
            y, nmr = emit_rstd_all(b)
            for sc in range(4):
                emit_apply(b, sc, y[:, sc : sc + 1], nmr[:, sc : sc + 1])

        # one merged Newton rstd chain for batches 0 and 1 ([128, 8]),
        # emitted once batch 1's bn stats are all aggregated
        chain01 = {}

        def emit_rstd01():
            # seed on DVE (reciprocal_approx_fast is DVE-only), the ~13-op
            # Newton chain on the otherwise-idle GPSIMD - DVE is the pacer
            mvb = mvall[:, 0:16].rearrange("p (c two) -> p c two", two=2)
            va = stp.tile([128, 8], F32, tag="va8", name="va8")
            nc.vector.tensor_scalar(va[:], mvb[:, :, 1:2], eps_t[:], 0.0, AX.add, AX.add)
            y = stp.tile([128, 8], F32, tag="ny8", name="ny8")
            nc.vector.reciprocal_approx_fast(y[:], va[:])
            cm5 = stp.tile([128, 8], F32, tag="cm5", name="cm5")
            nc.gpsimd.memset(cm5[:], -0.5)
            c15 = stp.tile([128, 8], F32, tag="c15", name="c15")
            nc.gpsimd.memset(c15[:], 1.5)
            for _ in range(3):
                t1 = stp.tile([128, 8], F32, tag="nt8", name="nt8")
                nc.gpsimd.tensor_tensor(t1[:], y[:], y[:], AX.mult)
                nc.gpsimd.tensor_tensor(t1[:], t1[:], va[:], AX.mult)
                nc.gpsimd.tensor_tensor(t1[:], t1[:], cm5[:], AX.mult)
                nc.gpsimd.tensor_tensor(t1[:], t1[:], c15[:], AX.add)
                nc.gpsimd.tensor_tensor(y[:], y[:], t1[:], AX.mult)
            nmr = stp.tile([128, 8], F32, tag="nm8", name="nmr8")
            nc.vector.scalar_tensor_tensor(
                nmr[:], mvb[:, :, 0:1], -1.0, y[:], AX.mult, AX.mult
            )
            chain01["y"] = y
            chain01["nmr"] = nmr

        def emit_apply01(b):
            y, nmr = chain01["y"], chain01["nmr"]
            for sc in range(4):
                j = 4 * b + sc
                emit_apply(b, sc, y[:, j : j + 1], nmr[:, j : j + 1])

        # ---- main schedule -------------------------------------------------
        # qproj for batch 0 first (only needs wqT + xT chunk 0)
        emit_qproj(0, 0)
        emit_qproj(1, 0)
        # bridge dummies: keep the PE busy between qproj and the first logits
        for w in range(8):
            wp = lgp.tile([128, 512], F32, tag="lg", name="bridge")
            mm(wp[:], warm_t[:, 0:128], warm_t[:], start=True, stop=True)

        def emit_av(b, g):
            emit_av_half(4 * b + g, 0)
            emit_av_half(4 * b + g, 1)

        # epilogue pieces of batch b-1, spread across batch b's groups.
        # LN applies for batches 0/1 are deferred to the merged chain01
        # (emitted in batch 3's slots); batch 2 uses the ACT-sqrt path
        # (legal: its g=3 slot runs after the final Exp op).
        def emit_epilogue_piece(b, g):
            if b < 0:
                return
            if g == 0:
                # DVE-light: the recip chain runs while PE streams the next
                # group, so the B matmuls at g=1 never stall the PE queue
                emit_norm_recip(b)
            elif g == 1:
                for k in range(4):
                    emit_norm_mult(b, k)
                emit_fc_chunk(b, 0)
            elif g == 2:
                emit_fc_chunk(b, 1)
                emit_fc_chunk(b, 2)
            else:
                emit_fc_chunk(b, 3)
                if b == 2:
                    emit_ln_sqrt(2, [0, 1, 2, 3])

        for b in range(BS):
            for g in range(4):
                emit_logits_exp_pair(b, g, 0)
                if b < BS - 1 and g == 1:
                    emit_qproj(0, b + 1)
                emit_logits_exp_pair(b, g, 1)
                if b < BS - 1 and g == 2:
                    emit_qproj(1, b + 1)
                # AV of the previous group (pipelined one group back)
                if g > 0:
                    emit_av(b, g - 1)
                elif b > 0:
                    emit_av(b - 1, 3)
                    emit_sums_dma(b - 1)
                if b == BS - 1 and g == 3:
                    # last AV pulled forward so the PE reaches it before the
                    # batch-2 epilogue matmuls; sums of groups 0-2 extracted
                    # early so their norm runs during the final AV
                    emit_sums_dma(b, (0, 1, 2))
                    emit_av(b, 3)
                    emit_tail_sums3(b)
                emit_epilogue_piece(b - 1, g)
                if b == BS - 1:
                    if g == 0:
                        emit_rstd01()
                    elif g == 1:
                        emit_apply01(0)
                    elif g == 2:
                        emit_apply01(1)
        # tail: last batch epilogue, ACT-sqrt LN per chunk
        bl = BS - 1
        emit_norm_recip(bl, 0, 6)
        for k in range(3):
            emit_norm_mult(bl, k)
        emit_tail_norm3(bl)
        for sc in range(4):
            emit_fc_chunk(bl, sc)
            emit_ln_sqrt_tail(bl, sc)

    nc.compile()
    return nc


def _prep_core(x, z, fc_b, v):
    """Build the per-core input map (host-side layout packing) for vert v."""
    bf = ml_dtypes.bfloat16
    xv = np.ascontiguousarray(x[:, :, v, :]).reshape(R, D_IN)  # [r, d]
    xT = np.ascontiguousarray(xv.T).astype(bf).reshape(2, 128, R)  # [d, r]
    xres = np.ascontiguousarray(
        (xv + fc_b[None, :]).reshape(16, 128, 256).transpose(1, 0, 2).reshape(128, 16 * 256)
    )
    zv = z.reshape(N_HEAD, NV, BS, SEG, D_Q)[:, v]  # [h, b, t, d]
    # zT carries the logits operand, pre-scaled so logits == log2(exp arg):
    # t = (q . z) * log2e / sqrt(dq)
    zTp = (zv * np.float32(LOG2E * INV_TEMP)).transpose(0, 1, 3, 2)  # [h, b, d, t]
    zT = np.zeros((2, 4, 32, 4, 512), bf)
    for o, h in enumerate(H_PERM):
        for b in range(BS):
            zT[o // 4, o % 4, :, b] = zTp[h, b]
    zT = np.ascontiguousarray(zT.reshape(2, 128, 2048))
    # zA: per av-pair slot q = 4b+g: [headA(g) 33 | headB(g+4) 33]
    zA = np.zeros((4, 128, 66 * 16), bf)
    za_full = np.concatenate(
        [zv, np.ones((N_HEAD, BS, SEG, 1), np.float32)], axis=-1
    ).astype(bf)  # [h, b, t, 33]
    for b in range(BS):
        for g in range(4):
            q = 4 * b + g
            for c in range(4):
                zA[c, :, 66 * q : 66 * q + 33] = za_full[g, b, 128 * c : 128 * (c + 1), :]
                zA[c, :, 66 * q + 33 : 66 * q + 66] = za_full[
                    g + 4, b, 128 * c : 128 * (c + 1), :
                ]
    return {"xT": xT, "xres": xres, "zT": zT, "zA": zA}


def kernel(x, z, w_q_w, w_q_b, fc_w, fc_b, ln_gamma, ln_beta, _trace=False, _tmpdir=None):
    from concourse.bass_utils import run_bass_kernel_spmd

    x = np.asarray(x, np.float32)
    z = np.asarray(z, np.float32)
    w_q_w = np.asarray(w_q_w, np.float32)
    w_q_b = np.asarray(w_q_b, np.float32)
    fc_w = np.asarray(fc_w, np.float32)
    fc_b = np.asarray(fc_b, np.float32)
    ln_gamma = np.asarray(ln_gamma, np.float32)
    ln_beta = np.asarray(ln_beta, np.float32)

    use_wqb = bool(np.any(w_q_b != 0.0))
    use_gb = bool(np.any(ln_gamma != 1.0) or np.any(ln_beta != 0.0))

    key = (use_wqb, use_gb)
    if key not in _prog_cache:
        _prog_cache[key] = _build(use_wqb, use_gb)
    nc = _prog_cache[key]

    bf = ml_dtypes.bfloat16
    # e' permutation: tile T col j -> head H_PERM[4T + j//32], dq j%32
    eperm = np.zeros(256, np.int64)
    for T in range(2):
        for j in range(128):
            o = 4 * T + j // 32
            eperm[128 * T + j] = 32 * H_PERM[o] + j % 32
    wqT = np.ascontiguousarray(w_q_w.T[:, eperm]).astype(bf).reshape(2, 128, 256)
    wqb_p = np.zeros((128, 2), np.float32)
    for T in range(2):
        wqb_p[:, T] = w_q_b[eperm[128 * T : 128 * (T + 1)]]
    # fcT tile k: rows 0:32 = fc rows of head k, rows 64:96 = head k+4, else 0
    fcT_full = fc_w.T  # [e, d_in]
    fcT = np.zeros((4, 128, D_IN), np.float32)
    for k in range(4):
        fcT[k, 0:32] = fcT_full[32 * k : 32 * (k + 1)]
        fcT[k, 64:96] = fcT_full[32 * (k + 4) : 32 * (k + 5)]
    fcT = fcT.astype(bf)
    # ind8 tile: B_k = ind8[:, 128k:128k+128].T @ recb
    ind8 = np.zeros((8, 512), bf)
    for k in range(4):
        ind8[2 * k, 128 * k : 128 * k + 32] = 1.0
        ind8[2 * k + 1, 128 * k + 64 : 128 * k + 96] = 1.0
    shared = {
        "wqT": wqT,
        "wqb": wqb_p,
        "ind8": ind8,
        "fcT": fcT,
        "gbb": np.ascontiguousarray(
            np.concatenate(
                [
                    np.broadcast_to(ln_gamma, (128, 256)),
                    np.broadcast_to(ln_beta, (128, 256)),
                ],
                axis=1,
            )
        ),
    }
    in_maps = []
    for v in range(NCORES):
        m = dict(shared)
        m.update(_prep_core(x, z, fc_b, v))
        in_maps.append(m)

    res = run_bass_kernel_spmd(
        nc,
        in_maps,
        core_ids=list(range(NCORES)),
        trace=_trace,
        tmpdir=_tmpdir,
    )
    out = np.empty((BS, SEG, NV, D_IN), np.float32)
    for v in range(NCORES):
        out[:, :, v, :] = res.results[v]["y"].reshape(BS, SEG, D_IN)
    kernel._last_result = res
    return out
